# revision 36
# baseline (speedup 1.0000x reference)
"""Gaussian tile rasterizer on 8 Trainium2 NeuronCores (Bass/Tile).

Problem config (hardcoded): 16384 gaussians, 512x512x3 image, 16px tiles
-> 1024 tiles, K=64 gaussians/tile, fp32.

Sharding: tile axis across 8 cores (128 tiles = 64 image rows per core);
gaussian arrays are reduced host-side to per-tile packed parameters.

Device math per tile (k = depth-ordered slot, p = local pixel):
  q'(k,p) = -0.5*quad + ln(opac)   as a rank-6 matmul  W[6,k]^T @ U[6,p]
            (U = [x^2, x, y^2, y, x*y, 1] over the 16x16 local grid; W folds
             the covariance inverse, local mean, opacity, and -0.5 scale;
             tf32 hi/lo split for fp32 accuracy at fp32r matmul speed)
  alpha   = exp(clip(q', ln .01, ln .99))          (clip in log domain)
  l       = ln(1 - alpha)
  lw      = Tri^T @ l      (strict-lower-triangular matmul = prefix sum)
  aw      = alpha * exp(lw)
  color   = aw^T @ tcol    (tcol zeroed on invalid slots, which makes the
            alpha valid-mask unnecessary: invalid slots only trail valid ones)
"""
import os
import sys
import time

import numpy as np

if '/opt/trn_rl_repo' not in sys.path:
    sys.path.insert(0, '/opt/trn_rl_repo')

N_GAUSS = 16384
IMG_W = 512
IMG_H = 512
T_LEN = 16
K_MAX = 64
Tx = IMG_W // T_LEN
Ty = IMG_H // T_LEN
T_TILES = Tx * Ty                       # 1024
N_CORES = 8
TILES_PER_CORE = T_TILES // N_CORES     # 128
PACKS = TILES_PER_CORE // 2             # 64 packs of 2 tiles
SUPER = PACKS // 2                      # 32 super-packs of 4 tiles

LOG_LO = float(np.log(np.float32(0.01)))
LOG_HI = float(np.log(np.float32(0.99)))


# ---------------------------------------------------------------- host prep
def _select(pos2d, cov2d, opacity, color, depth):
    """Depth-sort + per-tile first-K selection (identical to reference)."""
    a = cov2d[:, 0, 0]; b = cov2d[:, 0, 1]; c = cov2d[:, 1, 1]
    trace = a + c
    det = a * c - b * b
    term1 = np.float32(0.5) * trace
    term2 = np.float32(0.5) * np.sqrt(np.clip(trace * trace - np.float32(4.0) * det,
                                              np.float32(0.0), None))
    radius = np.float32(3.0) * np.sqrt(np.maximum(term1 - term2, term1 + term2))

    order = np.argsort(depth, kind='stable')
    pos2d = pos2d[order]; cov2d = cov2d[order]
    opacity = opacity[order]; color = color[order]; radius = radius[order]

    px = pos2d[:, 0]; py = pos2d[:, 1]; r = radius
    tx_lo = np.maximum(((px - r) * np.float32(1.0 / T_LEN)).astype(np.int32) - 1, 0)
    tx_hi = np.minimum(((px + r) * np.float32(1.0 / T_LEN)).astype(np.int32) + 1, Tx - 1)
    ty_lo = np.maximum(((py - r) * np.float32(1.0 / T_LEN)).astype(np.int32) - 1, 0)
    ty_hi = np.minimum(((py + r) * np.float32(1.0 / T_LEN)).astype(np.int32) + 1, Ty - 1)
    nxm = int((tx_hi - tx_lo).max()) + 1
    nym = int((ty_hi - ty_lo).max()) + 1

    gx = tx_lo[:, None] + np.arange(nxm, dtype=np.int32)[None, :]
    gy = ty_lo[:, None] + np.arange(nym, dtype=np.int32)[None, :]
    Lf = (gx << 4).astype(np.float32)
    Tf = (gy << 4).astype(np.float32)
    okx = (gx <= tx_hi[:, None]) \
        & (px[:, None] + r[:, None] > Lf) & (px[:, None] - r[:, None] < Lf + T_LEN)
    oky = (gy <= ty_hi[:, None]) \
        & (py[:, None] + r[:, None] > Tf) & (py[:, None] - r[:, None] < Tf + T_LEN)

    ok = okx[:, :, None] & oky[:, None, :]
    tid = gx[:, :, None] * np.int32(Ty) + gy[:, None, :]
    gidx, ii, jj = np.nonzero(ok)
    tids = tid[gidx, ii, jj]
    perm = np.argsort(tids, kind='stable')   # keeps depth order within tile
    tids_s = tids[perm]; g_s = gidx[perm]
    counts_full = np.bincount(tids_s, minlength=T_TILES)
    offs = np.zeros(T_TILES + 1, dtype=np.int64)
    np.cumsum(counts_full, out=offs[1:])
    slot = np.arange(tids_s.shape[0], dtype=np.int64) - offs[tids_s]
    keep = slot < K_MAX
    sel = np.zeros((T_TILES, K_MAX), dtype=np.int64)
    sel[tids_s[keep], slot[keep]] = g_s[keep]
    counts = np.minimum(counts_full, K_MAX)
    valid = np.arange(K_MAX)[None, :] < counts[:, None]
    return pos2d, cov2d, opacity, color, sel, valid


def _host_prep(pos2d, cov2d, opacity, color, depth):
    pos2d, cov2d, opacity, color, sel, valid = _select(
        pos2d, cov2d, opacity, color, depth)

    lefts = np.repeat(np.arange(Tx, dtype=np.float32) * T_LEN, Ty)
    tops = np.tile(np.arange(Ty, dtype=np.float32) * T_LEN, Tx)

    covs = cov2d[sel]
    ga = covs[:, :, 0, 0]; gb = covs[:, :, 0, 1]; gc = covs[:, :, 1, 1]
    inv = np.float32(1.0) / (ga * gc - gb * gb)
    A = gc * inv; B = gb * inv; C = ga * inv
    ps = pos2d[sel]
    mx = ps[:, :, 0] - lefts[:, None]
    my = ps[:, :, 1] - tops[:, None]
    lno = np.log(np.maximum(opacity[sel], np.float32(1e-30)))

    W = np.empty((T_TILES, 6, K_MAX), np.float32)
    W[:, 0] = A
    W[:, 1] = np.float32(-2.0) * A * mx + np.float32(2.0) * B * my
    W[:, 2] = C
    W[:, 3] = np.float32(-2.0) * C * my + np.float32(2.0) * B * mx
    W[:, 4] = np.float32(-2.0) * B
    W[:, 5] = A * mx * mx + C * my * my - np.float32(2.0) * B * mx * my \
        - np.float32(2.0) * lno
    W *= np.float32(-0.5)
    bad = ~valid
    W[:, 5][bad] = np.float32(-50.0)
    W[:, :5] *= valid[:, None, :]

    tcol = color[sel]
    tcol[bad] = 0.0

    # global sharded arrays: axis 0 is core-major (shard c = rows c*6 / c*64)
    wg = np.ascontiguousarray(
        W.reshape(N_CORES, TILES_PER_CORE, 6, K_MAX)
        .transpose(0, 2, 1, 3).reshape(N_CORES * 6, TILES_PER_CORE * K_MAX))
    # compact colors [64, PACKS*2*3] per core: (k, (m, half, c)); device
    # scatters into the block-diagonal [128, PACKS*6] layout
    tg = np.ascontiguousarray(
        tcol.reshape(N_CORES, PACKS, 2, K_MAX, 3)
        .transpose(0, 3, 1, 2, 4).reshape(N_CORES * K_MAX, PACKS * 6))
    return {"w": wg, "tca": tg}


def _u_basis():
    i = np.arange(T_LEN, dtype=np.float32)
    gi, gj = np.meshgrid(i, i, indexing='ij')
    x = gi.ravel(); y = gj.ravel()
    return np.ascontiguousarray(
        np.stack([x * x, x, y * y, y, x * y, np.ones_like(x)], axis=0))


def _tri_basis():
    t64 = np.triu(np.ones((K_MAX, K_MAX), np.float32), 1)   # [j,k]: j<k
    tri = np.zeros((2 * K_MAX, 2 * K_MAX), np.float32)
    tri[:K_MAX, :K_MAX] = t64
    tri[K_MAX:, K_MAX:] = t64
    return tri


# ---------------------------------------------------------------- device code
_CACHE = {}


def _split_multiwait(nc, mybir):
    """Walrus in this toolchain rejects >1 sync wait on Drain instructions;
    split extra waits onto single-wait NoOps executed just before."""
    f = nc.m.functions[0]
    for bb in f.blocks:
        insts = list(bb.instructions)
        changed = False
        out = []
        for inst in insts:
            si = inst.sync_info
            if si is not None and si.on_wait is not None and len(si.on_wait) > 1:
                waits = list(si.on_wait)
                for i, w in enumerate(waits[:-1]):
                    out.append(mybir.InstNoOp(
                        name=f"{inst.name}_waitsplit{i}",
                        engine=inst.engine,
                        sync_info=mybir.SyncInfo(on_wait=[w], on_update=[]),
                    ))
                si.on_wait = [waits[-1]]
                changed = True
            out.append(inst)
        if changed:
            bb.instructions = out


def _build_nc(split=True):
    import concourse.bass as bass
    import concourse.mybir as mybir
    from concourse.tile import TileContext

    f32 = mybir.dt.float32
    f32r = mybir.dt.float32r
    f16 = mybir.dt.float16
    bf16 = mybir.dt.bfloat16
    u32 = mybir.dt.uint32
    AF = mybir.ActivationFunctionType
    OP = mybir.AluOpType

    nc = bass.Bass()
    w_d = nc.dram_tensor("w", [6, PACKS * 128], f32, kind="ExternalInput")
    tca_d = nc.dram_tensor("tca", [K_MAX, PACKS * 6], f32, kind="ExternalInput")
    tri_d = nc.dram_tensor("tri", [128, 128], f32r, kind="ExternalInput")
    uc_d = nc.dram_tensor("uc", [6, 256], f32, kind="ExternalInput")
    # raw block dump [blk=(txl,h), pixel(i8,j16), (ty,c)]; host reassembles
    img_d = nc.dram_tensor("img", [8, 128, 96], f32, kind="ExternalOutput")

    with TileContext(nc) as tc:
        with (
            tc.tile_pool(name="const", bufs=1) as cpool,
            tc.tile_pool(name="work", bufs=3) as wpool,
            tc.tile_pool(name="qps", bufs=2, space="PSUM") as qpool,
            tc.tile_pool(name="lws", bufs=2, space="PSUM") as lwpool,
            tc.tile_pool(name="cols", bufs=1, space="PSUM") as colpool,
        ):
            w_s = cpool.tile([6, PACKS * 128], f32, tag="w")
            tcols_s = cpool.tile([128, PACKS * 6], f32, tag="tcols")
            tri_s = cpool.tile([128, 128], f32r, tag="tri")
            uc_s = cpool.tile([6, 256], f32, tag="uc")
            nc.sync.dma_start(out=w_s[:], in_=w_d[:])
            nc.vector.memset(tcols_s[:], 0.0)
            tca_v = tca_d.rearrange("k (m h c) -> h k m c", m=PACKS, h=2, c=3)
            tcv = tcols_s[:].rearrange("(h k) (m hh c) -> h k m hh c", h=2, m=PACKS, hh=2, c=3)
            nc.sync.dma_start(out=tcv[0, :, :, 0, :], in_=tca_v[0])
            nc.sync.dma_start(out=tcv[1, :, :, 1, :], in_=tca_v[1])
            nc.sync.dma_start(out=tri_s[:], in_=tri_d[:])
            nc.sync.dma_start(out=uc_s[:], in_=uc_d[:])

            colp = colpool.tile([128, 1024], f32, tag="colp")

            for sp in range(SUPER):
                qp = qpool.tile([128, 512], f32, tag="qp")
                for half in range(2):
                    m = 2 * sp + half
                    out_ap = qp[:, half * 256:(half + 1) * 256]
                    nc.tensor.matmul(
                        out_ap,
                        lhsT=w_s[:, m * 128:(m + 1) * 128],
                        rhs=uc_s[:],
                        start=True, stop=True)
                qc = wpool.tile([128, 512], f32, tag="qc")
                nc.vector.tensor_scalar(qc[:], qp[:], LOG_HI, LOG_LO,
                                        op0=OP.min, op1=OP.max)
                al = wpool.tile([128, 512], f32, tag="al")
                nc.scalar.activation(al[:], qc[:], AF.Exp)
                ll = wpool.tile([128, 512], f32r, tag="ll")
                nc.scalar.activation(ll[:], al[:], AF.Ln, bias=1.0, scale=-1.0)
                lwp = lwpool.tile([128, 512], f32, tag="lwp")
                for half in range(2):
                    s = half * 256
                    nc.tensor.matmul(
                        lwp[:, s:s + 256],
                        lhsT=tri_s[:],
                        rhs=ll[:, s:s + 256],
                        start=True, stop=True)
                ww = wpool.tile([128, 512], f32, tag="ww")
                nc.scalar.activation(ww[:], lwp[:], AF.Exp)
                aw = wpool.tile([128, 512], f32, tag="aw")
                nc.vector.tensor_mul(aw[:], al[:], ww[:])
                for half in range(2):
                    m = 2 * sp + half
                    txl = (2 * m) // 32
                    ty0 = (2 * m) % 32
                    for h in range(2):
                        off = (txl * 2 + h) * 128 + ty0 * 3
                        nc.tensor.matmul(
                            colp[:, off:off + 6],
                            lhsT=aw[:, half * 256 + h * 128:
                                    half * 256 + (h + 1) * 128],
                            rhs=tcols_s[:, m * 6:(m + 1) * 6],
                            start=True, stop=True)

            colsb = cpool.tile([128, 1024], f32, tag="colsb")
            for txl in range(4):
                for h in range(2):
                    blk = txl * 2 + h
                    off = blk * 128
                    nc.vector.tensor_copy(colsb[:, off:off + 96],
                                          colp[:, off:off + 96])
                    nc.sync.dma_start(out=img_d[blk],
                                      in_=colsb[:, off:off + 96])

    if split:
        _split_multiwait(nc, mybir)

    # Normalize debug info so the serialized BIR (and thus every compile
    # cache key) is independent of the directory kernel.py runs from.
    try:
        import orjson
        _orig_tjb = nc.to_json_bytes

        def _scrub(o):
            if isinstance(o, dict):
                if "filename" in o:
                    o["filename"] = "k.py"
                if "lineno" in o:
                    o["lineno"] = 0
                if "ant_traceback" in o:
                    o["ant_traceback"] = ""
                for v in o.values():
                    _scrub(v)
            elif isinstance(o, list):
                for v in o:
                    _scrub(v)

        def _to_json_bytes_scrubbed():
            bir = orjson.loads(_orig_tjb())
            _scrub(bir)
            return orjson.dumps(bir)

        nc.to_json_bytes = _to_json_bytes_scrubbed
    except Exception:
        pass
    return nc


def _get_runtime():
    """Build the Bass module once and a cached jitted SPMD runner.

    First call goes through bass_utils.run_bass_kernel_spmd (the standard
    entry; compiles the NEFF). Subsequent calls reuse a jitted shard_map
    callable (same lowering) to skip per-call retracing, keep the constant
    tensors device-resident, and recycle the donated output buffer.
    """
    if "rt" in _CACHE:
        return _CACHE["rt"]

    import jax
    cache_dir = os.environ.get("BASS_JAX_CACHE_DIR",
                               os.path.expanduser("~/.cache/jax_bass_cache"))
    try:
        os.makedirs(cache_dir, exist_ok=True)
        jax.config.update("jax_compilation_cache_dir", cache_dir)
        jax.config.update("jax_persistent_cache_min_entry_size_bytes", -1)
        jax.config.update("jax_persistent_cache_min_compile_time_secs", 0.0)
    except Exception:
        pass

    import concourse.mybir as mybir
    from concourse import bass2jax
    from concourse.bass_utils import run_bass_kernel_spmd
    from jax.sharding import Mesh, PartitionSpec, NamedSharding

    nc = _build_nc()

    partition_name = (nc.partition_id_tensor.name
                      if nc.partition_id_tensor else None)
    in_names, out_names, out_avals, zero_shapes = [], [], [], []
    for alloc in nc.m.functions[0].allocations:
        if not isinstance(alloc, mybir.MemoryLocationSet):
            continue
        name = alloc.memorylocations[0].name
        if alloc.kind == "ExternalInput":
            if name != partition_name:
                in_names.append(name)
        elif alloc.kind == "ExternalOutput":
            shape = tuple(alloc.tensor_shape)
            dtype = mybir.dt.np(alloc.dtype)
            out_names.append(name)
            out_avals.append(jax.core.ShapedArray(shape, dtype))
            zero_shapes.append((shape, dtype))
    n_params = len(in_names)
    n_outs = len(out_avals)
    all_in_names = list(in_names) + list(out_names) \
        + ([partition_name] if partition_name else [])
    donate = tuple(range(n_params, n_params + n_outs))

    def _body(*args):
        operands = list(args)
        if partition_name is not None:
            operands.append(bass2jax.partition_id_tensor())
        outs = bass2jax._bass_exec_p.bind(
            *operands, out_avals=tuple(out_avals), in_names=tuple(all_in_names),
            out_names=tuple(out_names), lowering_input_output_aliases=(),
            sim_require_finite=True, sim_require_nnan=True, nc=nc)
        return tuple(outs)

    devices = jax.devices()[:N_CORES]
    mesh = Mesh(np.asarray(devices), ("core",))
    in_specs = (PartitionSpec("core"),) * (n_params + n_outs)
    out_specs = (PartitionSpec("core"),) * n_outs
    sharded = jax.jit(
        bass2jax.shard_map(_body, mesh=mesh, in_specs=in_specs,
                           out_specs=out_specs, check_rep=False),
        donate_argnums=donate, keep_unused=True)
    shardspec = NamedSharding(mesh, PartitionSpec("core"))

    from concurrent.futures import ThreadPoolExecutor
    state = {"first": True, "prev_out": None, "const_dev": None,
             "pool": ThreadPoolExecutor(N_CORES)}

    def run(gl, uc, tri):
        if state["first"]:
            state["first"] = False
            in_maps = [{"w": gl["w"][c * 6:(c + 1) * 6],
                        "tca": gl["tca"][c * K_MAX:(c + 1) * K_MAX],
                        "tri": tri, "uc": uc} for c in range(N_CORES)]
            res = run_bass_kernel_spmd(nc, in_maps, list(range(N_CORES)))
            return [res.results[c]["img"] for c in range(N_CORES)]

        if state["const_dev"] is None:
            state["const_dev"] = {
                "tri": jax.device_put(
                    np.concatenate([tri] * N_CORES, axis=0), shardspec),
                "uc": jax.device_put(
                    np.concatenate([uc] * N_CORES, axis=0), shardspec),
            }
        cd = state["const_dev"]
        args = []
        for n in in_names:
            args.append(cd[n] if n in cd else gl[n])
        # donated output operand: recycle last call's output buffer (the
        # kernel overwrites every element, so contents are irrelevant)
        prev = state["prev_out"]
        for i, (s, d) in enumerate(zero_shapes):
            if prev is not None and not prev[i].is_deleted():
                args.append(prev[i])
            else:
                args.append(jax.device_put(
                    np.zeros((N_CORES * s[0], *s[1:]), d), shardspec))
        prof = os.environ.get("BASS_KERNEL_PROF")
        if prof:
            td0 = time.perf_counter()
        out_arrs = sharded(*args)
        if prof:
            td1 = time.perf_counter()
            jax.block_until_ready(out_arrs)
            td2 = time.perf_counter()
        state["prev_out"] = list(out_arrs)
        shards = sorted(out_arrs[0].addressable_shards,
                        key=lambda s: s.device.id)
        datas = list(state["pool"].map(lambda s: np.asarray(s.data), shards))
        res = datas
        if prof:
            td3 = time.perf_counter()
            print(f"[prof] dispatch {1e3*(td1-td0):.0f}ms "
                  f"exec-wait {1e3*(td2-td1):.0f}ms fetch {1e3*(td3-td2):.0f}ms",
                  flush=True)
        return res

    _CACHE["rt"] = run
    return run


# ---------------------------------------------------------------- entry point
def kernel(pos2d, cov2d, opacity, color, depth, width=IMG_W, height=IMG_H,
           tile_length=T_LEN, max_per_tile=K_MAX):
    assert int(width) == IMG_W and int(height) == IMG_H
    assert int(tile_length) == T_LEN and int(max_per_tile) == K_MAX

    pos2d = np.ascontiguousarray(pos2d, np.float32)
    cov2d = np.ascontiguousarray(cov2d, np.float32)
    opacity = np.ascontiguousarray(opacity, np.float32)
    color = np.ascontiguousarray(color, np.float32)
    depth = np.ascontiguousarray(depth, np.float32)

    run = _get_runtime()
    gl = _host_prep(pos2d, cov2d, opacity, color, depth)
    uc = _u_basis()
    tri = _tri_basis()
    imgs = run(gl, uc, tri)
    blocks = np.stack(imgs, axis=0)
    # [core, blk(txl,h), p(i8,j16), f(ty,c)] -> [512, 512, 3]
    img = blocks.reshape(N_CORES, 4, 2, 8, 16, 32, 3) \
        .transpose(0, 1, 2, 3, 5, 4, 6).reshape(IMG_W, IMG_H, 3)
    return np.ascontiguousarray(img, np.float32)
